# revision 28
# baseline (speedup 1.0000x reference)
"""AnchorSelfAttention1d Trainium2 kernel.

Data-parallel over batch: 8 samples -> 8 NeuronCores, one sample per core.
Per-core math for sample x [D=512, L=8192] (channels-first):
  qT/kT stay in [d, l] layout (lhsT = W^T tiles, rhs = x) -> no transposes.
  v computed directly in [l, d] layout (lhsT = x tiles, rhs = Wv^T).
  anchors: adaptive-avg-pool commutes with the linear ->
     xp = pool(x) via DVE segment reduces, aT = Wk^T.T @ xp + bk.
  stage1 logits SL1[p, l] = aT.T @ kT chunks; exp fused with the a.bk bias
     term on ScalarE (accum_out gives rowsums for free);
     e1 chunks PE-transposed ([100,128] blocks) for the c matmul.
  stage2 logits SL2[p, l] = aT.T @ qT chunks; e2T kept in [p, l];
     column sums via ones-vector matmul; normalization by 1/colsum is
     applied to the final output tiles (PE k=1 broadcast + DVE multiply).
  c = (sum_l e1T.T @ v) * r1[p] + bv  (bias-add exactness: sum_l s1 = 1).
  out rT[d, l] = c.T @ e2T * recip_colsum  -> already in output layout.

x and the weights are cast to bf16 on load (gpsimd cast-DMA) and stay
resident in SBUF; every DMA writes a fresh tile so no DMA ever needs more
than one semaphore wait (walrus sync-wait limit). All heavy matmuls run
bf16 with fp32 PSUM accumulation; softmax weights / v / c stored bf16.
"""

import sys

import numpy as np

for _p in ("/opt/trn_rl_repo",):
    if _p not in sys.path:
        sys.path.insert(0, _p)

import concourse.bass as bass  # noqa: E402
import concourse.tile as tile  # noqa: E402
from concourse import mybir  # noqa: E402
from concourse.bass_utils import run_bass_kernel_spmd  # noqa: E402
from concourse.masks import make_identity  # noqa: E402

B, D, L, P = 8, 512, 8192, 100
NG = D // 128  # 4 partition tiles of the d axis
LCH = 512  # l-chunk width for phase 1/3
NCH = L // LCH  # 16
LT = 128  # l-tile for phase 2 (v + c accumulation)
NLT = L // LT  # 64
MEG = 2048  # x load granularity (25 pool segments exactly)
NMEG = L // MEG  # 4
SEG_PER_MEG = P // NMEG  # 25

F32 = mybir.dt.float32
BF16 = mybir.dt.bfloat16
EXP = mybir.ActivationFunctionType.Exp
COPY = mybir.ActivationFunctionType.Copy
AX = mybir.AxisListType.X
ADD = mybir.AluOpType.add


def _segments():
    idx = np.arange(P)
    starts = (idx * L) // P
    ends = -((-(idx + 1) * L) // P)
    return starts, ends


def _squash_waits(nc):
    """Transitive semaphore-wait reduction.

    Tile emits per-proc-minimal waits but does not track that syncing on
    engine X transitively conveys X's own observed clock.  Walrus enforces
    a small per-instruction sync-command budget (matmul/DMA: ~1 wait), so
    we replay the scheduled streams with full vector clocks and drop any
    wait that is already implied by an earlier kept wait on the same
    engine.  Per-sem update order equals issue order (engine sems are
    single-writer; DMA lane sems are FIFO per issuing engine), so the
    snapshot attribution is sound.
    """
    insts = []
    for blk in nc.m.functions[0].blocks:
        insts.extend(blk.instructions)

    # sems with any non-additive update (barrier sub/reset) are untrackable
    unsafe = set()
    for inst in insts:
        si = inst.sync_info
        if si is None:
            continue
        for u in si.on_update:
            if u.sync_type != "semaphore" or u.update_mode not in (
                "sem-inc", "sem-add-imm"
            ):
                unsafe.add(u.id)

    know = {}  # engine -> {sem_id: known value}
    sem_val = {}  # sem_id -> cumulative value
    events = {}  # sem_id -> list of (value, snapshot dict)
    import bisect

    n_drop = 0
    for inst in insts:
        si = inst.sync_info
        eng = inst.engine
        k = know.setdefault(eng, {})
        if si is not None:
            clean = all(
                w.sync_type == "semaphore" and w.wait_mode == "sem-ge-imm"
                and w.wait_reg is None and w.id not in unsafe
                for w in si.on_wait
            )
            def _snap(w):
                if w.id in unsafe:
                    return {}
                evs = events.get(w.id, [])
                pos = bisect.bisect_left(evs, w.wait_value, key=lambda t: t[0])
                return evs[pos][1] if pos < len(evs) else {w.id: w.wait_value}

            order = sorted(
                si.on_wait, key=lambda w: len(_snap(w)), reverse=True
            )
            kept = []
            for w in order:
                if clean and k.get(w.id, 0) >= w.wait_value:
                    n_drop += 1
                    continue
                kept.append(w)
                for s_id, v in _snap(w).items():
                    if k.get(s_id, 0) < v:
                        k[s_id] = v
                if k.get(w.id, 0) < w.wait_value:
                    k[w.id] = w.wait_value
            if clean and len(kept) > 1:
                # fixpoint: drop any wait implied by the others' snapshots
                changed = True
                while changed and len(kept) > 1:
                    changed = False
                    for w in list(kept):
                        others = {}
                        for o in kept:
                            if o is w:
                                continue
                            for s_id, v in _snap(o).items():
                                if others.get(s_id, 0) < v:
                                    others[s_id] = v
                        if others.get(w.id, 0) >= w.wait_value:
                            kept.remove(w)
                            n_drop += 1
                            changed = True
                            break
            if clean and len(kept) < len(si.on_wait):
                inst.sync_info = type(si)(
                    on_wait=kept, on_update=list(si.on_update)
                )
            # process updates: snapshot = this instruction's knowledge
            for u in si.on_update:
                if u.sync_type != "semaphore" or u.id in unsafe:
                    continue
                val = sem_val.get(u.id, 0) + (u.update_value or 1)
                sem_val[u.id] = val
                snap = dict(k)
                snap[u.id] = val
                events.setdefault(u.id, []).append((val, snap))
    return n_drop


def build_nc(phases=3):
    nc = bass.Bass()
    x_e = nc.declare_dram_parameter("x", [D, L], F32, isOutput=False)
    wqt_e = nc.declare_dram_parameter("wqt", [D, D], F32, isOutput=False)
    wkt_e = nc.declare_dram_parameter("wkt", [D, D], F32, isOutput=False)
    wvt_e = nc.declare_dram_parameter("wvt", [D, D], F32, isOutput=False)
    bq_e = nc.declare_dram_parameter("bq", [D], F32, isOutput=False)
    bk_e = nc.declare_dram_parameter("bk", [D], F32, isOutput=False)
    bv_e = nc.declare_dram_parameter("bv", [D], F32, isOutput=False)
    lrec_e = nc.declare_dram_parameter("lrec", [NG * P], F32, isOutput=False)
    out_e = nc.declare_dram_parameter("out", [D, L], F32, isOutput=True)

    starts, ends = _segments()
    xr = x_e.rearrange("(g p) l -> p g l", p=128)

    with tile.TileContext(nc) as tc:
        with (
            tc.tile_pool(name="glob", bufs=1) as glob,
            tc.tile_pool(name="gpsum", bufs=1, space="PSUM") as gpsum,
        ):
            # ---------------- constants + weights (bf16 cast loads) ------
            x_sb = glob.tile([128, NG, L], BF16)
            wqt = glob.tile([128, NG, D], BF16)
            wkt = glob.tile([128, NG, D], BF16)
            wvt = glob.tile([128, NG, D], BF16)
            bqt = glob.tile([128, NG], F32)
            bkt = glob.tile([128, NG], F32)
            lrbc = glob.tile([128, NG * P], BF16)
            bvbc = glob.tile([128, D], BF16)
            ident = glob.tile([128, 128], BF16)
            ones_bf = glob.tile([128, 1], BF16)
            ones1f = glob.tile([1, 128], F32)
            xps = glob.tile([128, NG, P], F32)
            xps_bf = glob.tile([128, NG, P], BF16)
            aT_bf = glob.tile([128, NG, P], BF16)
            bqtb = glob.tile([128, NG], BF16)
            bktb = glob.tile([128, NG], BF16)
            abk_sb = glob.tile([128, 1], F32)
            abq_sb = glob.tile([128, 1], F32)
            rs1 = glob.tile([128, NCH], F32)
            r1 = glob.tile([128, 1], F32)
            e1T = glob.tile([128, NLT, P], BF16)
            e2T = glob.tile([128, L], BF16)
            cs1d = glob.tile([1, L], BF16)
            ones1b = glob.tile([1, 128], BF16)
            scr = glob.tile([1, 8], F32)
            lrt = scr[:, 0:1]
            lrt2 = scr[:, 1:2]
            act_t = scr[:, 2:3]
            rb1 = scr[:, 3:4]
            rb2 = scr[:, 4:5]
            pt = scr[:, 5:6]
            bcs_all = glob.tile([128, NCH, LCH], BF16)
            c_sb = glob.tile([128, D], BF16)

            for w_sb, w_ext in ((wqt, wqt_e), (wkt, wkt_e), (wvt, wvt_e)):
                nc.gpsimd.dma_start(
                    out=w_sb, in_=w_ext.rearrange("(g p) d -> p g d", p=128)
                )
            nc.sync.dma_start(out=bqt, in_=bq_e.rearrange("(g p) -> p g", p=128))
            nc.sync.dma_start(out=bkt, in_=bk_e.rearrange("(g p) -> p g", p=128))
            lrec_ap = lrec_e[:]
            nc.gpsimd.dma_start(
                out=lrbc,
                in_=bass.AP(
                    tensor=lrec_ap.tensor, offset=lrec_ap.offset,
                    ap=[[0, 128]] + list(lrec_ap.ap),
                ),
            )
            bv_ap = bv_e[:]
            nc.gpsimd.dma_start(
                out=bvbc,
                in_=bass.AP(
                    tensor=bv_ap.tensor, offset=bv_ap.offset,
                    ap=[[0, 128]] + list(bv_ap.ap),
                ),
            )
            nc.scalar.activation(bqtb, bqt, COPY)
            nc.scalar.activation(bktb, bkt, COPY)
            make_identity(nc, ident)
            nc.vector.memset(ones_bf, 1.0)
            nc.vector.memset(ones1b, 1.0)
            nc.vector.memset(ones1f, 1.0)

            # x load: one cast-DMA (f32 HBM -> bf16 SBUF, 16 SDMA engines)
            nc.gpsimd.dma_start(out=x_sb, in_=xr)
            for i in range(P):
                s, e = int(starts[i]), int(ends[i])
                nc.vector.tensor_reduce(
                    out=xps[:, :, i : i + 1], in_=x_sb[:, :, s:e],
                    axis=AX, op=ADD,
                )
            nc.vector.tensor_copy(lrt, lrbc[0:1, 0:1])
            nc.vector.tensor_copy(lrt2, bvbc[0:1, 0:1])
            xps_f = xps.rearrange("p g s -> p (g s)")
            nc.vector.tensor_mul(xps_f, xps_f, lrbc)
            nc.scalar.activation(
                xps_bf.rearrange("p g s -> p (g s)"), xps_f, COPY
            )

            dum_ps = gpsum.tile([1, 64], F32, tag="dum")
            dum_n = [0]

            def observe(*aps):
                # tiny PE matmuls so the PE clock covers these tensors'
                # DMA lanes before real consumers (walrus 1-wait limit)
                for ap in aps:
                    i = dum_n[0]
                    dum_n[0] += 1
                    nc.tensor.matmul(
                        dum_ps[:, i % 64 : i % 64 + 1], lhsT=ap, rhs=ap,
                        start=True, stop=True,
                    )

            if phases >= 1:
                # ---------- phase 1: kT/qT chunks; SL1/SL2; exps ----------
                with (
                    tc.tile_pool(name="p1", bufs=10) as p1,
                    tc.tile_pool(name="p1e", bufs=2) as p1e,
                    tc.tile_pool(name="p1p", bufs=2, space="PSUM") as p1p,
                    tc.tile_pool(name="p1tp", bufs=2, space="PSUM") as p1tp,
                ):
                    def kq_chunk(c):
                        l0 = c * LCH
                        res = []
                        for w_sb in (wkt, wqt):
                            t_sb = p1.tile([128, NG, LCH], BF16, tag="ktq")
                            for g in range(NG):
                                ps = p1p.tile([128, LCH], F32, tag="ktqp")
                                for kg in range(NG):
                                    nc.tensor.matmul(
                                        ps,
                                        lhsT=w_sb[:, kg, g * 128 : (g + 1) * 128],
                                        rhs=x_sb[:, kg, l0 : l0 + LCH],
                                        start=(kg == 0),
                                        stop=(kg == NG - 1),
                                    )
                                nc.scalar.activation(t_sb[:, g, :], ps, COPY)
                            res.append(t_sb)
                        return res

                    observe(
                        wkt[0:1, 0, 0:1], wqt[0:1, 0, 0:1],
                        x_sb[0:1, 0, 0:1], bkt[0:1, 0:1], bqt[0:1, 0:1],
                    )
                    WARM = 5
                    kq = {c: kq_chunk(c) for c in range(WARM)}

                    # anchors: aT = Wk^T.T @ xp + bk ; bias scalars a.bk/a.bq
                    for g in range(NG):
                        aps = gpsum.tile([128, P], F32, tag="ap")
                        for kg in range(NG):
                            nc.tensor.matmul(
                                aps,
                                lhsT=wkt[:, kg, g * 128 : (g + 1) * 128],
                                rhs=xps_bf[:, kg, :],
                                start=(kg == 0),
                                stop=(kg == NG - 1),
                            )
                        nc.vector.tensor_scalar_add(
                            aT_bf[:, g, :], aps, bkt[:, g : g + 1]
                        )
                    for dst, bt in ((abk_sb, bktb), (abq_sb, bqtb)):
                        abps = gpsum.tile([128, P], F32, tag="ap")
                        for g in range(NG):
                            nc.tensor.matmul(
                                abps[:P, :1],
                                lhsT=aT_bf[:, g, :],
                                rhs=bt[:, g : g + 1],
                                start=(g == 0),
                                stop=(g == NG - 1),
                            )
                        nc.scalar.activation(dst[:P, :], abps[:P, :1], COPY)

                    for c in range(NCH):
                        cn = c + WARM
                        if cn < NCH and cn not in kq:
                            kq[cn] = kq_chunk(cn)
                        l0 = c * LCH
                        kt, qt = kq.pop(c)

                        s1ps = p1p.tile([128, LCH], F32, tag="slp")
                        for g in range(NG):
                            nc.tensor.matmul(
                                s1ps[:P, :],
                                lhsT=aT_bf[:, g, :],
                                rhs=kt[:, g, :],
                                start=(g == 0),
                                stop=(g == NG - 1),
                            )
                        e1c = p1e.tile([128, LCH], BF16, tag="e1c")
                        nc.scalar.activation(
                            e1c[:P, :], s1ps[:P, :], EXP,
                            bias=abk_sb[:P, :], accum_out=rs1[:P, c : c + 1],
                        )
                        for j in range(4):
                            tps = p1tp.tile([128, P], BF16, tag="tp")
                            nc.tensor.transpose(
                                tps,
                                in_=e1c[:P, j * 128 : (j + 1) * 128],
                                identity=ident[:P, :P],
                            )
                            nc.vector.tensor_copy(e1T[:, c * 4 + j, :], tps)

                        s2ps = p1p.tile([128, LCH], F32, tag="slp")
                        for g in range(NG):
                            nc.tensor.matmul(
                                s2ps[:P, :],
                                lhsT=aT_bf[:, g, :],
                                rhs=qt[:, g, :],
                                start=(g == 0),
                                stop=(g == NG - 1),
                            )
                        nc.scalar.activation(
                            e2T[:P, l0 : l0 + LCH], s2ps[:P, :], EXP,
                            bias=abq_sb[:P, :],
                        )
                        csps = p1p.tile([1, LCH], F32, tag="slp")
                        nc.tensor.matmul(
                            csps, lhsT=ones_bf[:P, :],
                            rhs=e2T[:P, l0 : l0 + LCH],
                            start=True, stop=True,
                        )
                        nc.scalar.activation(cs1d[:, l0 : l0 + LCH], csps, COPY)

                nc.vector.tensor_reduce(
                    out=r1[:P, :], in_=rs1[:P, :], axis=AX, op=ADD
                )
                nc.vector.reciprocal(r1[:P, :], r1[:P, :])

            if phases >= 2:
                # ---------- phase 2: v tiles + c accumulation -------------
                with (
                    tc.tile_pool(name="p2", bufs=3) as p2,
                    tc.tile_pool(name="p2p", bufs=2, space="PSUM") as p2p,
                    tc.tile_pool(name="p2c", bufs=1, space="PSUM") as p2c,
                ):
                    observe(wvt[0:1, 0, 0:1])
                    cps = p2c.tile([128, D], F32, tag="cp")
                    for t in range(NLT):
                        l0 = t * LT
                        vps = p2p.tile([128, D], F32, tag="vp")
                        for kg in range(NG):
                            nc.tensor.matmul(
                                vps,
                                lhsT=x_sb[:, kg, l0 : l0 + LT],
                                rhs=wvt[:, kg, :],
                                start=(kg == 0),
                                stop=(kg == NG - 1),
                            )
                        vsb = p2.tile([128, D], BF16, tag="vs")
                        nc.scalar.activation(vsb, vps, COPY)
                        nc.tensor.matmul(
                            cps[:P, :],
                            lhsT=e1T[:, t, :],
                            rhs=vsb,
                            start=(t == 0),
                            stop=(t == NLT - 1),
                            skip_group_check=True,
                        )
                    nc.scalar.activation(act_t, cps[0:1, 0:1], COPY)
                    nc.scalar.activation(
                        c_sb[:P, :], cps[:P, :], COPY, scale=r1[:P, :]
                    )
                    nc.vector.tensor_add(c_sb[:P, :], c_sb[:P, :], bvbc[:P, :])

            if phases >= 3:
                # ---------- phase 3: normalize e2T; out = c.T @ e2T ------
                with (
                    tc.tile_pool(name="p3r", bufs=1) as p3r,
                    tc.tile_pool(name="p3p", bufs=3, space="PSUM") as p3p,
                    tc.tile_pool(name="p3bp", bufs=2, space="PSUM") as p3bp,
                ):
                    rt = p3r.tile([128, NG, L], BF16, tag="rt")
                    # POOL observes ACT (e2T fully written)
                    nc.gpsimd.tensor_copy(pt, e2T[0:1, L - 1 : L])
                    # PE observes DVE (c_sb final add)
                    observe(c_sb[0:1, 0:1])
                    for c in range(NCH):
                        l0 = c * LCH
                        bcp = p3bp.tile([128, LCH], F32, tag="bcp")
                        nc.tensor.matmul(
                            bcp, lhsT=ones1b, rhs=cs1d[:, l0 : l0 + LCH],
                            start=True, stop=True,
                        )
                        with nc.allow_low_precision("bf16 norm reciprocals"):
                            nc.vector.reciprocal(bcs_all[:, c, :], bcp)
                        nc.gpsimd.tensor_mul(
                            e2T[:P, l0 : l0 + LCH], e2T[:P, l0 : l0 + LCH],
                            bcs_all[:P, c, :],
                        )
                    for g in range(NG):
                        for c in range(NCH):
                            l0 = c * LCH
                            if g == 0:
                                observe(e2T[0:1, l0 : l0 + 1])
                            rps = p3p.tile([128, LCH], F32, tag="rp")
                            nc.tensor.matmul(
                                rps,
                                lhsT=c_sb[:P, g * 128 : (g + 1) * 128],
                                rhs=e2T[:P, l0 : l0 + LCH],
                                start=True, stop=True,
                            )
                            nc.scalar.activation(
                                rt[:, g, l0 : l0 + LCH], rps, COPY
                            )
                    nc.gpsimd.dma_start(
                        out=out_e.rearrange("(g p) l -> p g l", p=128),
                        in_=rt,
                    )
    _squash_waits(nc)
    return nc


_NC = None


def _get_nc():
    global _NC
    if _NC is None:
        _NC = build_nc()
    return _NC


def _host_inputs(Wq, bq, Wk, bk, Wv, bv):
    starts, ends = _segments()
    lens = (ends - starts).astype(np.float32)
    lrec = np.tile(1.0 / lens, NG).astype(np.float32)
    return {
        "wqt": np.ascontiguousarray(np.asarray(Wq, np.float32).T),
        "wkt": np.ascontiguousarray(np.asarray(Wk, np.float32).T),
        "wvt": np.ascontiguousarray(np.asarray(Wv, np.float32).T),
        "bq": np.asarray(bq, np.float32),
        "bk": np.asarray(bk, np.float32),
        "bv": np.asarray(bv, np.float32),
        "lrec": lrec,
    }


def run(x, Wq, bq, Wk, bk, Wv, bv, trace=False):
    nc = _get_nc()
    x = np.asarray(x, np.float32)
    shared = _host_inputs(Wq, bq, Wk, bk, Wv, bv)
    in_maps = [
        {"x": np.ascontiguousarray(x[i]), **shared} for i in range(x.shape[0])
    ]
    res = run_bass_kernel_spmd(
        nc, in_maps, core_ids=list(range(len(in_maps))), trace=trace
    )
    out = np.stack([res.results[i]["out"] for i in range(len(in_maps))])
    return out.astype(np.float32), res


def kernel(x, Wq, bq, Wk, bk, Wv, bv):
    out, _ = run(x, Wq, bq, Wk, bk, Wv, bv, trace=False)
    return out
